# revision 46
# baseline (speedup 1.0000x reference)
"""Additive (Bahdanau) cross-attention kernel for 8 TRN2 NeuronCores.

Math: scores[b,q,k] = sum_h v[h] * tanh(qh[b,q,h] + kh[b,k,h]),
      weights = softmax_k(scores), out = weights @ values,
      returns (out, weights) like the reference.

Algorithm: tanh(z) ~= sum_j b_j sin(w_j z) (5-term Fourier sine fit on
|z|<=5, max err 1.1e-2), and sin(w(qh+kh)) = sin(w qh)cos(w kh) +
cos(w qh)sin(w kh), so the O(B*LQ*LK*H) tanh work becomes 2J rank-H
TensorEngine matmuls plus O((LQ+LK)*H) ACT-engine sin/cos evaluations.
The device Sin spline is only valid for |x| <~ pi, so only harmonics
1,2,3 are evaluated directly (args <= 3.05); 4 and 6 come from exact
double-angle products on the VectorEngine (GAMMA compensates the 1/2
per doubling). Scores are O(1) so softmax needs no max-subtraction, and
exp uses the fused accumulator for row sums.

Sharding: batch (4) x query-half (2) -> 8 cores; keys/values replicated
per batch pair; no collectives. Host packs shards in bf16 and in
per-128-column-chunk transposed layout (pure byte permutation) so the
d-contraction matmuls need no on-device transposes; inputs are spread
over the three DMA rings (sync / scalar HWDGE + gpsimd SWDGE) so the
keys, q-side, and values pipelines all start as early as possible.

Hardware quirks honored:
- Most instructions encode at most ONE semaphore wait (regular matmuls
  two); excess waits (notably Tile's kernel-tail drain) are hoisted
  into preceding same-engine Drain instructions by a post-pass over the
  serialized BIR (_split_excess_waits).
- PE transposes (S3_LW) are kept to single-engine dependencies.
- GpSimd shares an exclusive SBUF port with the VectorEngine: long
  gpsimd SBUF ops stall DVE, so gpsimd is used only for DMA issue.
- ACT sin and exp live in different table sets: all sins are ordered
  before the exps (add_dep_helper) so exactly two table loads happen.
"""

import numpy as np
import ml_dtypes
from contextlib import ExitStack

import concourse.bass as bass
import concourse.mybir as mybir
import concourse.tile as tile
from concourse.bass_utils import run_bass_kernel_spmd

B, LQ, LK, D, H = 4, 256, 1024, 512, 128
QS = LQ // 2      # 128 queries per core
NCORE = 8
DCH = D // 128    # 4 contraction chunks
KT = LK // 128    # 8 key tiles

# tanh(z) ~= sum_j BCOEF[j] * sin(GRID[j]*OMEGA1*z); maxerr 2.7e-2, rms@data
# 1.6e-3.  Only GRID 1,2,3 are evaluated by the ACT Sin table (args stay
# within its accurate |x|<~pi window); 4, 6, 8 come from exact double-angle
# products.  GAMMA[j] compensates the 1/2-per-doubling in the product tiles.
OMEGA1 = 0.41887902047863906
GRID = [1, 2, 3, 4, 6]
BCOEF = [1.1408133594, 0.0791848126, 0.143795538, 0.0857697298,
         0.0358456276]
GAMMA = {1: 1.0, 2: 1.0, 3: 1.0, 4: 2.0, 6: 2.0}
DERIVED = {4: 2, 6: 3}   # freq -> source freq (doubling)
J = len(GRID)
HALF_PI = 1.5707963267948966

f32 = mybir.dt.float32
bf16 = mybir.dt.bfloat16

_CACHE = {}


def _build():
    nc = bass.Bass("TRN2")
    # Inputs arrive pre-transposed per 128-column chunk (host-side layout
    # choice): qwa = [WkT | id | v | halfpi], qwb = [queryT | WqT],
    # keysT[:, c, k] = keys[k, c*128+p]. Values are loaded as bf16 by a
    # casting SWDGE DMA.
    d_qwc = nc.dram_tensor("qwc", [128, 2], f32, kind="ExternalInput")
    d_qwa = nc.dram_tensor("qwa", [128, D + 128], bf16, kind="ExternalInput")
    d_qwb = nc.dram_tensor("qwb", [128, 2 * D], bf16, kind="ExternalInput")
    d_keysT = nc.dram_tensor("keysT", [2, 128, DCH, 512], bf16, kind="ExternalInput")
    d_vals = nc.dram_tensor("values", [128, KT, D], bf16, kind="ExternalInput")
    d_wout = nc.dram_tensor("wout", [QS, LK], f32, kind="ExternalOutput")
    d_out = nc.dram_tensor("out", [QS, D], f32, kind="ExternalOutput")

    Sin = mybir.ActivationFunctionType.Sin
    Exp = mybir.ActivationFunctionType.Exp
    mult = mybir.AluOpType.mult
    add = mybir.AluOpType.add

    with tile.TileContext(nc) as tc, ExitStack() as ctx:
        const = ctx.enter_context(tc.tile_pool(name="const", bufs=1))
        ldp = ctx.enter_context(tc.tile_pool(name="ldp", bufs=2))
        persist = ctx.enter_context(tc.tile_pool(name="persist", bufs=1))
        harm_k = ctx.enter_context(tc.tile_pool(name="harm_k", bufs=1))
        harm_q = ctx.enter_context(tc.tile_pool(name="harm_q", bufs=1))
        tailp = ctx.enter_context(tc.tile_pool(name="tailp", bufs=1))
        ps_tr = ctx.enter_context(tc.tile_pool(name="ps_tr", bufs=2, space="PSUM"))
        ps_qh = ctx.enter_context(tc.tile_pool(name="ps_qh", bufs=1, space="PSUM"))
        ps_kh = ctx.enter_context(tc.tile_pool(name="ps_kh", bufs=2, space="PSUM"))
        ps_sc = ctx.enter_context(tc.tile_pool(name="ps_sc", bufs=2, space="PSUM"))
        ps_out = ctx.enter_context(tc.tile_pool(name="ps_out", bufs=1, space="PSUM"))

        # ---- input DMAs: all matrices pre-cast to bf16 host-side (the shard
        # storage format; identical rounding to an on-device cast) and spread
        # across the three DMA rings so keys, q-side, and values land早 ----
        keysT = [persist.tile([128, DCH, 512], bf16, tag=f"keysT{h}",
                              name=f"keysT{h}") for h in range(2)]
        nc.sync.dma_start(out=keysT[0][:, 0:2, :], in_=d_keysT[0][:, 0:2, :])
        nc.gpsimd.dma_start(out=keysT[0][:, 2:4, :], in_=d_keysT[0][:, 2:4, :])
        qwc_sb = const.tile([128, 2], f32, tag="qwc_sb")
        nc.scalar.dma_start(out=qwc_sb[:], in_=d_qwc[:])
        qwa_bf = const.tile([128, D + 128], bf16, tag="qwa_bf")
        nc.scalar.dma_start(out=qwa_bf[:], in_=d_qwa[:])
        nc.sync.dma_start(out=keysT[1][:, 0:2, :], in_=d_keysT[1][:, 0:2, :])
        qwb_bf = const.tile([128, 2 * D], bf16, tag="qwb_bf")
        nc.gpsimd.dma_start(out=qwb_bf[:], in_=d_qwb[:])
        nc.gpsimd.dma_start(out=keysT[1][:, 2:4, :], in_=d_keysT[1][:, 2:4, :])
        vals_bf = persist.tile([128, KT, D], bf16, tag="vals_bf")
        nc.sync.dma_start(out=vals_bf[:], in_=d_vals[:])

        # casts (DVE-owned per the single-wait transpose discipline)
        halfpi_ap = qwc_sb[:, 1:2]
        v_sb = const.tile([128, 1], f32, tag="v_sb")
        nc.scalar.copy(v_sb[:], qwc_sb[:, 0:1])
        WkT = qwa_bf[:, 0:D]
        # DVE-stamped identity so transposes keep a single-engine dep
        id_tile = const.tile([128, 128], bf16, tag="id_tile")
        nc.vector.tensor_copy(id_tile[:], qwa_bf[:, D:D + 128])
        id_bf = id_tile[:]
        queryT = qwb_bf[:, 0:D]
        WqT = qwb_bf[:, D:2 * D]

        def transpose_group(dst_copies, srcs):
            """PE-transpose up to 8 [128,128] bf16 blocks through one
            [128,1024] bf16 PSUM tile (one bank), freed by ONE DVE copy."""
            p = ps_tr.tile([128, 1024], bf16, tag="tr", name="tr_p")
            for i, src_ap in enumerate(srcs):
                nc.tensor.transpose(p[:, i * 128:(i + 1) * 128], src_ap, id_bf)
            dst_copies(p)

        # ---- projections ----
        qhT = ps_qh.tile([128, 128], f32, tag="qhT")
        for c in range(DCH):
            nc.tensor.matmul(qhT[:], WqT[:, c * 128:(c + 1) * 128],
                             queryT[:, c * 128:(c + 1) * 128],
                             start=(c == 0), stop=(c == DCH - 1))
        khTs = []
        for h in range(2):
            khT = ps_kh.tile([128, 512], f32, tag="khT", name=f"khT{h}")
            for c in range(DCH):
                nc.tensor.matmul(khT[:], WkT[:, c * 128:(c + 1) * 128],
                                 keysT[h][:, c, :], start=(c == 0),
                                 stop=(c == DCH - 1))
            khTs.append(khT)

        # ---- q-side harmonics (ACT sins + DVE ladder + folds) ----
        qt_s, qt_c = {}, {}
        for jf in (1, 2, 3):
            w = jf * OMEGA1
            s = harm_q.tile([128, 128], bf16, tag=f"sinq{jf}", name=f"sinq{jf}")
            nc.scalar.activation(s[:], qhT[:], Sin, bias=0.0, scale=w)
            c = harm_q.tile([128, 128], bf16, tag=f"cosq{jf}", name=f"cosq{jf}")
            nc.scalar.activation(c[:], qhT[:], Sin, bias=halfpi_ap, scale=w)
            qt_s[jf], qt_c[jf] = s, c
        for jf, sf in DERIVED.items():
            g2 = -2.0 * GAMMA[sf] * GAMMA[sf]
            s = harm_q.tile([128, 128], bf16, tag=f"sdq{jf}", name=f"sdq{jf}")
            nc.vector.tensor_tensor(s[:], qt_s[sf][:], qt_c[sf][:], mult)
            c = harm_q.tile([128, 128], bf16, tag=f"cdq{jf}", name=f"cdq{jf}")
            nc.vector.tensor_tensor(c[:], qt_s[sf][:], qt_s[sf][:], mult)
            nc.vector.tensor_scalar(c[:], c[:], float(g2), 1.0, mult, add)
            qt_s[jf], qt_c[jf] = s, c
        lhs_s, lhs_c = {}, {}
        for j, jf in enumerate(GRID):
            bg = float(BCOEF[j] * GAMMA[jf])
            ls = harm_q.tile([128, 128], bf16, tag=f"lhs_s{jf}", name=f"lhs_s{jf}")
            nc.vector.tensor_scalar(ls[:], qt_s[jf][:], v_sb[:], bg, mult, mult)
            lc = harm_q.tile([128, 128], bf16, tag=f"lhs_c{jf}", name=f"lhs_c{jf}")
            nc.vector.tensor_scalar(lc[:], qt_c[jf][:], v_sb[:], bg, mult, mult)
            lhs_s[jf], lhs_c[jf] = ls, lc

        # ---- per-half ACT trig + DVE ladder + score matmuls ----
        scores = [ps_sc.tile([128, 512], f32, tag="scores", name=f"scores{i}")
                  for i in range(2)]
        exp_f = tailp.tile([128, LK], f32, tag="exp_f")
        exp_bf = tailp.tile([128, LK], bf16, tag="exp_bf")
        sums = [tailp.tile([128, 1], f32, tag=f"sum{kh}", name=f"sum{kh}")
                for kh in range(2)]
        last_sin = [None]

        def half_harmonics(h):
            khT = khTs[h]
            kt_s, kt_c = {}, {}
            for jf in (1, 2, 3):
                w = jf * OMEGA1
                s = harm_k.tile([128, 512], bf16, tag=f"sink{jf}_{h}",
                                name=f"sink{jf}_{h}")
                nc.scalar.activation(s[:], khT[:], Sin, bias=0.0, scale=w)
                c = harm_k.tile([128, 512], bf16, tag=f"cosk{jf}_{h}",
                                name=f"cosk{jf}_{h}")
                last_sin[0] = nc.scalar.activation(c[:], khT[:], Sin,
                                                   bias=halfpi_ap, scale=w)
                kt_s[jf], kt_c[jf] = s, c
            for jf, sf in DERIVED.items():
                g2 = -2.0 * GAMMA[sf] * GAMMA[sf]
                s = harm_k.tile([128, 512], bf16, tag=f"sdk{jf}_{h}",
                                name=f"sdk{jf}_{h}")
                nc.vector.tensor_tensor(s[:], kt_s[sf][:], kt_c[sf][:], mult)
                c = harm_k.tile([128, 512], bf16, tag=f"cdk{jf}_{h}",
                                name=f"cdk{jf}_{h}")
                nc.vector.tensor_tensor(c[:], kt_s[sf][:], kt_s[sf][:], mult)
                nc.vector.tensor_scalar(c[:], c[:], float(g2), 1.0, mult, add)
                kt_s[jf], kt_c[jf] = s, c
            for j, jf in enumerate(GRID):
                nc.tensor.matmul(scores[h][:], lhs_s[jf][:], kt_c[jf][:],
                                 start=(j == 0), stop=False)
                nc.tensor.matmul(scores[h][:], lhs_c[jf][:], kt_s[jf][:],
                                 start=False, stop=(j == J - 1))

        half_harmonics(0)
        half_harmonics(1)

        # ---- softmax + tail (exps after all sins: one ACT table switch) ----
        from concourse.tile import add_dep_helper
        outp = ps_out.tile([128, D], f32, tag="outp")
        for h in range(2):
            sl = slice(h * 512, (h + 1) * 512)
            ei = nc.scalar.activation(exp_f[:, sl], scores[h][:], Exp, bias=0.0,
                                      scale=1.0, accum_out=sums[h][:])
            add_dep_helper(ei.ins, last_sin[0].ins, sync=False,
                           reason="exp after all sins (one table switch)")
            nc.vector.tensor_copy(exp_bf[:, sl], exp_f[:, sl])
            wT = tailp.tile([128, 512], bf16, tag=f"wT{h}", name=f"wT{h}")
            transpose_group(
                lambda p, wT=wT: nc.vector.tensor_copy(wT[:], p[:, :512]),
                [exp_bf[:, h * 512 + i * 128:h * 512 + (i + 1) * 128]
                 for i in range(4)])
            for i in range(4):
                t = h * 4 + i
                nc.tensor.matmul(outp[:], wT[:, i * 128:(i + 1) * 128],
                                 vals_bf[:, t, :], start=(t == 0),
                                 stop=(t == KT - 1))

        sumtot = tailp.tile([128, 1], f32, tag="sumtot")
        nc.vector.tensor_tensor(sumtot[:], sums[0][:], sums[1][:], add)
        recip = tailp.tile([128, 1], f32, tag="recip")
        nc.vector.reciprocal(recip[:], sumtot[:])
        wf_sb = tailp.tile([128, LK], f32, tag="wf_sb")
        nc.vector.tensor_scalar(wf_sb[:, 0:512], exp_f[:, 0:512], recip[:],
                                None, mult)
        nc.scalar.dma_start(out=d_wout[:, 0:512], in_=wf_sb[:, 0:512])
        nc.vector.tensor_scalar(wf_sb[:, 512:], exp_f[:, 512:], recip[:],
                                None, mult)
        nc.sync.dma_start(out=d_wout[:, 512:], in_=wf_sb[:, 512:])
        out_sb = tailp.tile([128, D], f32, tag="out_sb")
        nc.vector.tensor_scalar(out_sb[:], outp[:], recip[:], None, mult)
        nc.scalar.dma_start(out=d_out[:, 0:256], in_=out_sb[:, 0:256])
        nc.sync.dma_start(out=d_out[:, 256:], in_=out_sb[:, 256:])

    return nc


def _wait_limit(inst):
    op = inst.get("opcode")
    if op == "Matmult":
        return 1 if inst.get("is_transpose") else 2
    return 1


def _split_excess_waits(raw):
    """Walrus enforces tiny per-instruction sync-wait budgets (1 for most ops,
    2 for Drain/regular Matmult). Tile sometimes emits more (notably the
    kernel-tail drain, which waits on every engine + DMA lane). Hoist the
    excess into preceding same-engine Drain instructions."""
    import json as _json
    d = _json.loads(raw)
    n_split = 0
    for fn in d.get("functions", []):
        for bb in fn.get("blocks", []):
            insts = bb.get("instructions", [])
            out = []
            for inst in insts:
                si = inst.get("sync_info") or {}
                waits = si.get("on_wait") or []
                lim = _wait_limit(inst)
                if len(waits) > lim:
                    excess, keep = waits[:-lim], waits[-lim:]
                    for i, wcmd in enumerate(excess):
                        n_split += 1
                        out.append({
                            "debug": inst.get("debug"),
                            "engine": inst["engine"],
                            "ins": [], "outs": [],
                            "name": f"{inst['name']}-ws{i}",
                            "opcode": "Drain",
                            "sync_info": {"on_wait": [wcmd]},
                        })
                    si["on_wait"] = keep
                    inst["sync_info"] = si
                out.append(inst)
            bb["instructions"] = out
    return _json.dumps(d).encode()


def _patch_json(nc):
    orig = nc.to_json_bytes

    def patched():
        return _split_excess_waits(orig())

    nc.to_json_bytes = patched


def _get_nc():
    if "nc" not in _CACHE:
        nc = _build()
        _patch_json(nc)
        _CACHE["nc"] = nc
    return _CACHE["nc"]


def _chunkT(m):
    """[128, D] -> per-128-column-chunk transpose: out[:, c*128:(c+1)*128] =
    m[:, c*128:(c+1)*128].T  (pure layout permutation for the shard)."""
    return np.concatenate([m[:, c * 128:(c + 1) * 128].T
                           for c in range(m.shape[1] // 128)], axis=1)


def _run(inputs, trace=False):
    nc = _get_nc()
    return _run_with_retry(nc, inputs, trace)


def _run_with_retry(nc, inputs, trace):
    query = np.asarray(inputs["query"], dtype=np.float32)
    keys = np.asarray(inputs["keys"], dtype=np.float32)
    values = np.asarray(inputs["values"], dtype=np.float32)
    Wq = np.ascontiguousarray(np.asarray(inputs["Wq"], dtype=np.float32))
    Wk = np.ascontiguousarray(np.asarray(inputs["Wk"], dtype=np.float32))
    v = np.asarray(inputs["v"], dtype=np.float32)

    in_maps = []
    for c in range(NCORE):
        b, qh = c // 2, c % 2
        bf = ml_dtypes.bfloat16
        qs = query[b, qh * QS:(qh + 1) * QS, :]
        qwc = np.concatenate(
            [v.reshape(H, 1), np.full((128, 1), HALF_PI, np.float32)], axis=1)
        qwa = np.concatenate(
            [_chunkT(Wk), np.eye(128, dtype=np.float32)], axis=1).astype(bf)
        qwb = np.concatenate([_chunkT(qs), _chunkT(Wq)], axis=1).astype(bf)
        kT = np.stack(
            [np.stack([keys[b][h * 512:(h + 1) * 512, c * 128:(c + 1) * 128].T
                       for c in range(DCH)], axis=1) for h in range(2)]
        ).astype(bf)
        vT = values[b].reshape(KT, 128, D).transpose(1, 0, 2).astype(bf)
        in_maps.append({
            "qwc": np.ascontiguousarray(qwc),
            "qwa": np.ascontiguousarray(qwa),
            "qwb": np.ascontiguousarray(qwb),
            "keysT": np.ascontiguousarray(kT),
            "values": np.ascontiguousarray(vT),
        })
    res = None
    last_err = None
    for attempt in range(3):
        try:
            res = run_bass_kernel_spmd(nc, in_maps, core_ids=list(range(NCORE)),
                                       trace=trace)
            break
        except Exception as e:  # transient NRT/device faults: retry same NEFF
            last_err = e
            if attempt == 1:
                _CACHE.clear()
                nc = _get_nc()
    if res is None:
        raise last_err
    out = np.zeros((B, LQ, D), dtype=np.float32)
    wout = np.zeros((B, LQ, LK), dtype=np.float32)
    for c in range(NCORE):
        b, qh = c // 2, c % 2
        wout[b, qh * QS:(qh + 1) * QS, :] = res.results[c]["wout"]
        out[b, qh * QS:(qh + 1) * QS, :] = res.results[c]["out"]
    return (out, wout), res


def kernel(query, keys, values, Wq, Wk, v):
    (out, wout), _ = _run(dict(query=query, keys=keys, values=values,
                               Wq=Wq, Wk=Wk, v=v))
    return (out, wout)


# revision 47
# speedup vs baseline: 1.1479x; 1.1479x over previous
"""Additive (Bahdanau) cross-attention kernel for 8 TRN2 NeuronCores.

Math: scores[b,q,k] = sum_h v[h] * tanh(qh[b,q,h] + kh[b,k,h]),
      weights = softmax_k(scores), out = weights @ values,
      returns (out, weights) like the reference.

Algorithm: tanh(z) ~= sum_j b_j sin(w_j z) (5-term Fourier sine fit on
|z|<=5, max err 1.1e-2), and sin(w(qh+kh)) = sin(w qh)cos(w kh) +
cos(w qh)sin(w kh), so the O(B*LQ*LK*H) tanh work becomes 2J rank-H
TensorEngine matmuls plus O((LQ+LK)*H) ACT-engine sin/cos evaluations.
The device Sin spline is only valid for |x| <~ pi, so only harmonics
1,2,3 are evaluated directly (args <= 3.05); 4 and 6 come from exact
double-angle products on the VectorEngine (GAMMA compensates the 1/2
per doubling). Scores are O(1) so softmax needs no max-subtraction, and
exp uses the fused accumulator for row sums.

Sharding: batch (4) x query-half (2) -> 8 cores; keys/values replicated
per batch pair; no collectives. Host packs shards in bf16 and in
per-128-column-chunk transposed layout (pure byte permutation) so the
d-contraction matmuls need no on-device transposes; inputs are spread
over the three DMA rings (sync / scalar HWDGE + gpsimd SWDGE) so the
keys, q-side, and values pipelines all start as early as possible.

Hardware quirks honored:
- Most instructions encode at most ONE semaphore wait (regular matmuls
  two); excess waits (notably Tile's kernel-tail drain) are hoisted
  into preceding same-engine Drain instructions by a post-pass over the
  serialized BIR (_split_excess_waits).
- PE transposes (S3_LW) are kept to single-engine dependencies.
- GpSimd shares an exclusive SBUF port with the VectorEngine: long
  gpsimd SBUF ops stall DVE, so gpsimd is used only for DMA issue.
- ACT sin and exp live in different table sets: all sins are ordered
  before the exps (add_dep_helper) so exactly two table loads happen.
"""

import numpy as np
import ml_dtypes
from contextlib import ExitStack

import concourse.bass as bass
import concourse.mybir as mybir
import concourse.tile as tile
from concourse.bass_utils import run_bass_kernel_spmd

B, LQ, LK, D, H = 4, 256, 1024, 512, 128
QS = LQ // 2      # 128 queries per core
NCORE = 8
DCH = D // 128    # 4 contraction chunks
KT = LK // 128    # 8 key tiles

# tanh(z) ~= sum_j BCOEF[j] * sin(GRID[j]*OMEGA1*z); maxerr 2.7e-2, rms@data
# 1.6e-3.  Only GRID 1,2,3 are evaluated by the ACT Sin table (args stay
# within its accurate |x|<~pi window); 4, 6, 8 come from exact double-angle
# products.  GAMMA[j] compensates the 1/2-per-doubling in the product tiles.
OMEGA1 = 0.41887902047863906
GRID = [1, 2, 3, 4, 6]
BCOEF = [1.1408133594, 0.0791848126, 0.143795538, 0.0857697298,
         0.0358456276]
GAMMA = {1: 1.0, 2: 1.0, 3: 1.0, 4: 2.0, 6: 2.0}
DERIVED = {4: 2, 6: 3}   # freq -> source freq (doubling)
J = len(GRID)
HALF_PI = 1.5707963267948966

f32 = mybir.dt.float32
bf16 = mybir.dt.bfloat16

_CACHE = {}


def _build():
    nc = bass.Bass("TRN2")
    # Inputs arrive pre-transposed per 128-column chunk (host-side layout
    # choice): qwa = [WkT | id | v | halfpi], qwb = [queryT | WqT],
    # keysT[:, c, k] = keys[k, c*128+p]. Values are loaded as bf16 by a
    # casting SWDGE DMA.
    d_qwc = nc.dram_tensor("qwc", [128, 2], f32, kind="ExternalInput")
    d_qwa = nc.dram_tensor("qwa", [128, D + 128], bf16, kind="ExternalInput")
    d_qwb = nc.dram_tensor("qwb", [2, 128, D], bf16, kind="ExternalInput")
    d_keysT = nc.dram_tensor("keysT", [2, 128, DCH, 512], bf16, kind="ExternalInput")
    d_vals = nc.dram_tensor("values", [128, KT, D], bf16, kind="ExternalInput")
    d_wout = nc.dram_tensor("wout", [QS, LK], f32, kind="ExternalOutput")
    d_out = nc.dram_tensor("out", [QS, D], f32, kind="ExternalOutput")

    Sin = mybir.ActivationFunctionType.Sin
    Exp = mybir.ActivationFunctionType.Exp
    mult = mybir.AluOpType.mult
    add = mybir.AluOpType.add

    with tile.TileContext(nc) as tc, ExitStack() as ctx:
        const = ctx.enter_context(tc.tile_pool(name="const", bufs=1))
        ldp = ctx.enter_context(tc.tile_pool(name="ldp", bufs=2))
        persist = ctx.enter_context(tc.tile_pool(name="persist", bufs=1))
        harm_k = ctx.enter_context(tc.tile_pool(name="harm_k", bufs=1))
        harm_q = ctx.enter_context(tc.tile_pool(name="harm_q", bufs=1))
        tailp = ctx.enter_context(tc.tile_pool(name="tailp", bufs=1))
        ps_tr = ctx.enter_context(tc.tile_pool(name="ps_tr", bufs=2, space="PSUM"))
        ps_qh = ctx.enter_context(tc.tile_pool(name="ps_qh", bufs=1, space="PSUM"))
        ps_kh = ctx.enter_context(tc.tile_pool(name="ps_kh", bufs=2, space="PSUM"))
        ps_sc = ctx.enter_context(tc.tile_pool(name="ps_sc", bufs=2, space="PSUM"))
        ps_out = ctx.enter_context(tc.tile_pool(name="ps_out", bufs=1, space="PSUM"))

        # ---- input DMAs: all matrices pre-cast to bf16 host-side (the shard
        # storage format; identical rounding to an on-device cast) and spread
        # across the three DMA rings so keys, q-side, and values land早 ----
        keysT = [persist.tile([128, DCH, 512], bf16, tag=f"keysT{h}",
                              name=f"keysT{h}") for h in range(2)]
        qwb_bf = const.tile([128, 2 * D], bf16, tag="qwb_bf")
        nc.sync.dma_start(out=keysT[0][:, 0:2, :], in_=d_keysT[0][:, 0:2, :])
        nc.gpsimd.dma_start(out=keysT[0][:, 2:4, :], in_=d_keysT[0][:, 2:4, :])
        qwc_sb = const.tile([128, 2], f32, tag="qwc_sb")
        nc.scalar.dma_start(out=qwc_sb[:], in_=d_qwc[:])
        qwa_bf = const.tile([128, D + 128], bf16, tag="qwa_bf")
        nc.scalar.dma_start(out=qwa_bf[:], in_=d_qwa[:])
        nc.sync.dma_start(out=qwb_bf[:, 0:D], in_=d_qwb[0])
        nc.gpsimd.dma_start(out=qwb_bf[:, D:2 * D], in_=d_qwb[1])
        nc.sync.dma_start(out=keysT[1][:, 0:2, :], in_=d_keysT[1][:, 0:2, :])
        nc.gpsimd.dma_start(out=keysT[1][:, 2:4, :], in_=d_keysT[1][:, 2:4, :])
        vals_bf = persist.tile([128, KT, D], bf16, tag="vals_bf")
        nc.sync.dma_start(out=vals_bf[:], in_=d_vals[:])

        # casts (DVE-owned per the single-wait transpose discipline)
        halfpi_ap = qwc_sb[:, 1:2]
        v_sb = const.tile([128, 1], f32, tag="v_sb")
        nc.scalar.copy(v_sb[:], qwc_sb[:, 0:1])
        WkT = qwa_bf[:, 0:D]
        # DVE-stamped identity so transposes keep a single-engine dep
        id_tile = const.tile([128, 128], bf16, tag="id_tile")
        nc.vector.tensor_copy(id_tile[:], qwa_bf[:, D:D + 128])
        id_bf = id_tile[:]
        queryT = qwb_bf[:, 0:D]
        WqT = qwb_bf[:, D:2 * D]

        def transpose_group(dst_copies, srcs):
            """PE-transpose up to 8 [128,128] bf16 blocks through one
            [128,1024] bf16 PSUM tile (one bank), freed by ONE DVE copy."""
            p = ps_tr.tile([128, 1024], bf16, tag="tr", name="tr_p")
            for i, src_ap in enumerate(srcs):
                nc.tensor.transpose(p[:, i * 128:(i + 1) * 128], src_ap, id_bf)
            dst_copies(p)

        # ---- projections ----
        qhT = ps_qh.tile([128, 128], f32, tag="qhT")
        for c in range(DCH):
            nc.tensor.matmul(qhT[:], WqT[:, c * 128:(c + 1) * 128],
                             queryT[:, c * 128:(c + 1) * 128],
                             start=(c == 0), stop=(c == DCH - 1))
        khTs = []
        for h in range(2):
            khT = ps_kh.tile([128, 512], f32, tag="khT", name=f"khT{h}")
            for c in range(DCH):
                nc.tensor.matmul(khT[:], WkT[:, c * 128:(c + 1) * 128],
                                 keysT[h][:, c, :], start=(c == 0),
                                 stop=(c == DCH - 1))
            khTs.append(khT)

        # ---- q-side harmonics (ACT sins + DVE ladder + folds) ----
        qt_s, qt_c = {}, {}
        for jf in (1, 2, 3):
            w = jf * OMEGA1
            s = harm_q.tile([128, 128], bf16, tag=f"sinq{jf}", name=f"sinq{jf}")
            nc.scalar.activation(s[:], qhT[:], Sin, bias=0.0, scale=w)
            c = harm_q.tile([128, 128], bf16, tag=f"cosq{jf}", name=f"cosq{jf}")
            nc.scalar.activation(c[:], qhT[:], Sin, bias=halfpi_ap, scale=w)
            qt_s[jf], qt_c[jf] = s, c
        for jf, sf in DERIVED.items():
            g2 = -2.0 * GAMMA[sf] * GAMMA[sf]
            s = harm_q.tile([128, 128], bf16, tag=f"sdq{jf}", name=f"sdq{jf}")
            nc.vector.tensor_tensor(s[:], qt_s[sf][:], qt_c[sf][:], mult)
            c = harm_q.tile([128, 128], bf16, tag=f"cdq{jf}", name=f"cdq{jf}")
            nc.vector.tensor_tensor(c[:], qt_s[sf][:], qt_s[sf][:], mult)
            nc.vector.tensor_scalar(c[:], c[:], float(g2), 1.0, mult, add)
            qt_s[jf], qt_c[jf] = s, c
        lhs_s, lhs_c = {}, {}
        for j, jf in enumerate(GRID):
            bg = float(BCOEF[j] * GAMMA[jf])
            ls = harm_q.tile([128, 128], bf16, tag=f"lhs_s{jf}", name=f"lhs_s{jf}")
            nc.vector.tensor_scalar(ls[:], qt_s[jf][:], v_sb[:], bg, mult, mult)
            lc = harm_q.tile([128, 128], bf16, tag=f"lhs_c{jf}", name=f"lhs_c{jf}")
            nc.vector.tensor_scalar(lc[:], qt_c[jf][:], v_sb[:], bg, mult, mult)
            lhs_s[jf], lhs_c[jf] = ls, lc

        # ---- per-half ACT trig + DVE ladder + score matmuls ----
        scores = [ps_sc.tile([128, 512], f32, tag="scores", name=f"scores{i}")
                  for i in range(2)]
        exp_f = tailp.tile([128, LK], f32, tag="exp_f")
        exp_bf = tailp.tile([128, LK], bf16, tag="exp_bf")
        sums = [tailp.tile([128, 1], f32, tag=f"sum{kh}", name=f"sum{kh}")
                for kh in range(2)]
        last_sin = [None]

        def half_harmonics(h):
            khT = khTs[h]
            kt_s, kt_c = {}, {}
            for jf in (1, 2, 3):
                w = jf * OMEGA1
                s = harm_k.tile([128, 512], bf16, tag=f"sink{jf}_{h}",
                                name=f"sink{jf}_{h}")
                nc.scalar.activation(s[:], khT[:], Sin, bias=0.0, scale=w)
                c = harm_k.tile([128, 512], bf16, tag=f"cosk{jf}_{h}",
                                name=f"cosk{jf}_{h}")
                last_sin[0] = nc.scalar.activation(c[:], khT[:], Sin,
                                                   bias=halfpi_ap, scale=w)
                kt_s[jf], kt_c[jf] = s, c
            for jf, sf in DERIVED.items():
                g2 = -2.0 * GAMMA[sf] * GAMMA[sf]
                s = harm_k.tile([128, 512], bf16, tag=f"sdk{jf}_{h}",
                                name=f"sdk{jf}_{h}")
                nc.vector.tensor_tensor(s[:], kt_s[sf][:], kt_c[sf][:], mult)
                c = harm_k.tile([128, 512], bf16, tag=f"cdk{jf}_{h}",
                                name=f"cdk{jf}_{h}")
                nc.vector.tensor_tensor(c[:], kt_s[sf][:], kt_s[sf][:], mult)
                nc.vector.tensor_scalar(c[:], c[:], float(g2), 1.0, mult, add)
                kt_s[jf], kt_c[jf] = s, c
            for j, jf in enumerate(GRID):
                nc.tensor.matmul(scores[h][:], lhs_s[jf][:], kt_c[jf][:],
                                 start=(j == 0), stop=False)
                nc.tensor.matmul(scores[h][:], lhs_c[jf][:], kt_s[jf][:],
                                 start=False, stop=(j == J - 1))

        half_harmonics(0)
        half_harmonics(1)

        # ---- softmax + tail (exps after all sins: one ACT table switch) ----
        from concourse.tile import add_dep_helper
        outp = ps_out.tile([128, D], f32, tag="outp")
        for h in range(2):
            sl = slice(h * 512, (h + 1) * 512)
            ei = nc.scalar.activation(exp_f[:, sl], scores[h][:], Exp, bias=0.0,
                                      scale=1.0, accum_out=sums[h][:])
            add_dep_helper(ei.ins, last_sin[0].ins, sync=False,
                           reason="exp after all sins (one table switch)")
            nc.vector.tensor_copy(exp_bf[:, sl], exp_f[:, sl])
            wT = tailp.tile([128, 512], bf16, tag=f"wT{h}", name=f"wT{h}")
            transpose_group(
                lambda p, wT=wT: nc.vector.tensor_copy(wT[:], p[:, :512]),
                [exp_bf[:, h * 512 + i * 128:h * 512 + (i + 1) * 128]
                 for i in range(4)])
            for i in range(4):
                t = h * 4 + i
                nc.tensor.matmul(outp[:], wT[:, i * 128:(i + 1) * 128],
                                 vals_bf[:, t, :], start=(t == 0),
                                 stop=(t == KT - 1))

        sumtot = tailp.tile([128, 1], f32, tag="sumtot")
        nc.vector.tensor_tensor(sumtot[:], sums[0][:], sums[1][:], add)
        recip = tailp.tile([128, 1], f32, tag="recip")
        nc.vector.reciprocal(recip[:], sumtot[:])
        wf_sb = tailp.tile([128, LK], f32, tag="wf_sb")
        nc.vector.tensor_scalar(wf_sb[:, 0:512], exp_f[:, 0:512], recip[:],
                                None, mult)
        nc.scalar.dma_start(out=d_wout[:, 0:512], in_=wf_sb[:, 0:512])
        nc.vector.tensor_scalar(wf_sb[:, 512:], exp_f[:, 512:], recip[:],
                                None, mult)
        nc.sync.dma_start(out=d_wout[:, 512:], in_=wf_sb[:, 512:])
        out_sb = tailp.tile([128, D], f32, tag="out_sb")
        nc.vector.tensor_scalar(out_sb[:], outp[:], recip[:], None, mult)
        nc.scalar.dma_start(out=d_out[:, 0:256], in_=out_sb[:, 0:256])
        nc.sync.dma_start(out=d_out[:, 256:], in_=out_sb[:, 256:])

    return nc


def _wait_limit(inst):
    op = inst.get("opcode")
    if op == "Matmult":
        return 1 if inst.get("is_transpose") else 2
    return 1


def _split_excess_waits(raw):
    """Walrus enforces tiny per-instruction sync-wait budgets (1 for most ops,
    2 for Drain/regular Matmult). Tile sometimes emits more (notably the
    kernel-tail drain, which waits on every engine + DMA lane). Hoist the
    excess into preceding same-engine Drain instructions."""
    import json as _json
    d = _json.loads(raw)
    n_split = 0
    for fn in d.get("functions", []):
        for bb in fn.get("blocks", []):
            insts = bb.get("instructions", [])
            out = []
            for inst in insts:
                si = inst.get("sync_info") or {}
                waits = si.get("on_wait") or []
                lim = _wait_limit(inst)
                if len(waits) > lim:
                    excess, keep = waits[:-lim], waits[-lim:]
                    for i, wcmd in enumerate(excess):
                        n_split += 1
                        out.append({
                            "debug": inst.get("debug"),
                            "engine": inst["engine"],
                            "ins": [], "outs": [],
                            "name": f"{inst['name']}-ws{i}",
                            "opcode": "Drain",
                            "sync_info": {"on_wait": [wcmd]},
                        })
                    si["on_wait"] = keep
                    inst["sync_info"] = si
                out.append(inst)
            bb["instructions"] = out
    return _json.dumps(d).encode()


def _patch_json(nc):
    orig = nc.to_json_bytes

    def patched():
        return _split_excess_waits(orig())

    nc.to_json_bytes = patched


def _get_nc():
    if "nc" not in _CACHE:
        nc = _build()
        _patch_json(nc)
        _CACHE["nc"] = nc
    return _CACHE["nc"]


def _chunkT(m):
    """[128, D] -> per-128-column-chunk transpose: out[:, c*128:(c+1)*128] =
    m[:, c*128:(c+1)*128].T  (pure layout permutation for the shard)."""
    return np.concatenate([m[:, c * 128:(c + 1) * 128].T
                           for c in range(m.shape[1] // 128)], axis=1)


def _run(inputs, trace=False):
    nc = _get_nc()
    return _run_with_retry(nc, inputs, trace)


def _run_with_retry(nc, inputs, trace):
    query = np.asarray(inputs["query"], dtype=np.float32)
    keys = np.asarray(inputs["keys"], dtype=np.float32)
    values = np.asarray(inputs["values"], dtype=np.float32)
    Wq = np.ascontiguousarray(np.asarray(inputs["Wq"], dtype=np.float32))
    Wk = np.ascontiguousarray(np.asarray(inputs["Wk"], dtype=np.float32))
    v = np.asarray(inputs["v"], dtype=np.float32)

    in_maps = []
    for c in range(NCORE):
        b, qh = c // 2, c % 2
        bf = ml_dtypes.bfloat16
        qs = query[b, qh * QS:(qh + 1) * QS, :]
        qwc = np.concatenate(
            [v.reshape(H, 1), np.full((128, 1), HALF_PI, np.float32)], axis=1)
        qwa = np.concatenate(
            [_chunkT(Wk), np.eye(128, dtype=np.float32)], axis=1).astype(bf)
        qwb = np.stack([_chunkT(qs), _chunkT(Wq)]).astype(bf)
        kT = np.stack(
            [np.stack([keys[b][h * 512:(h + 1) * 512, c * 128:(c + 1) * 128].T
                       for c in range(DCH)], axis=1) for h in range(2)]
        ).astype(bf)
        vT = values[b].reshape(KT, 128, D).transpose(1, 0, 2).astype(bf)
        in_maps.append({
            "qwc": np.ascontiguousarray(qwc),
            "qwa": np.ascontiguousarray(qwa),
            "qwb": np.ascontiguousarray(qwb),
            "keysT": np.ascontiguousarray(kT),
            "values": np.ascontiguousarray(vT),
        })
    res = None
    last_err = None
    for attempt in range(3):
        try:
            res = run_bass_kernel_spmd(nc, in_maps, core_ids=list(range(NCORE)),
                                       trace=trace)
            break
        except Exception as e:  # transient NRT/device faults: retry same NEFF
            last_err = e
            if attempt == 1:
                _CACHE.clear()
                nc = _get_nc()
    if res is None:
        raise last_err
    out = np.zeros((B, LQ, D), dtype=np.float32)
    wout = np.zeros((B, LQ, LK), dtype=np.float32)
    for c in range(NCORE):
        b, qh = c // 2, c % 2
        wout[b, qh * QS:(qh + 1) * QS, :] = res.results[c]["wout"]
        out[b, qh * QS:(qh + 1) * QS, :] = res.results[c]["out"]
    return (out, wout), res


def kernel(query, keys, values, Wq, Wk, v):
    (out, wout), _ = _run(dict(query=query, keys=keys, values=values,
                               Wq=Wq, Wk=Wk, v=v))
    return (out, wout)
